# revision 33
# baseline (speedup 1.0000x reference)
"""Trainium2 Bass kernel for EruSelfAttentionModel.

Math (reference, simplified):
  e  = emb_table[x]                                  # [B,S,E] gather
  h  = LayerNorm(e) * gamma + beta                   # over E
  q  = einsum('hae,bse->bhsa', Wq, h); k likewise    # A=64 per head
  v  = einsum('hve,bse->bhsv', Wv, h)                # v-dim = E
  scores = q @ k^T / sqrt(E)
  sn = (scores - min) / (max - min)  (rowwise)
  softmax_sel = 1 - max(sn) == 0 exactly  =>  weights = sigmoid(10*sn - 5)
  out = weights @ v                                  # [B,H,S,E]

Key identities used:
  - sn is invariant to positive rescaling of scores => the 1/sqrt(E) scale
    can be dropped entirely.
  - weights = sigmoid(alpha * scores + beta_row) with per-row
    alpha = 10/(mx-mn), beta_row = -10*mn/(mx-mn) - 5  => single fused
    ScalarE activation pass (per-partition scale/bias APs).

Engine budget per core (span target ~210us):
  PE:   vhat 256 MMs + proj 64 + scores 128 (row-group pairs run
        concurrently) + out 512, all N=512 bf16 @ ~215ns  ~= 200us
  DVE:  scores PSUM->SBUF copy w/ min-accum (2x 1.2us/iter) + stats +
        one ob copy per iter + LN
  ACT:  sigmoids + vhat copies (phase A) + one ob copy per iter
  Pool: gathers + row-max pools (bf16 SBUF)
  Sync: h/w XBAR transposes + fused out DMAs + const loads

Sharding: data-parallel over batch; core b computes batch b fully.
"""

import os
import sys

sys.path.insert(0, "/opt/trn_rl_repo")

import numpy as np
import ml_dtypes

import concourse.bass as bass
import concourse.bacc as bacc
import concourse.tile as tile
from concourse import mybir
from concourse.bass_utils import run_bass_kernel_spmd

BF16 = ml_dtypes.bfloat16

VOCAB, E, A, H = 32000, 512, 64, 8
B, S = 8, 1024
P = 128                 # partitions
NCH = S // P            # 8 token chunks
EC = E // P             # 4 embedding chunks
LN_EPS = 1e-5

F32 = mybir.dt.float32
BF = mybir.dt.bfloat16
I16 = mybir.dt.int16

_BUILD_CACHE = {}
LAST_RESULTS = None     # test.py reads exec_time_ns from here


def build_nc(use_beta: bool):
    if (use_beta,) in _BUILD_CACHE:
        return _BUILD_CACHE[(use_beta,)]

    nc = bacc.Bacc("TRN2", target_bir_lowering=False, num_devices=8)

    idx_d = nc.declare_dram_parameter("idx", [P, S // 16], I16, isOutput=False)
    emb_d = nc.declare_dram_parameter("emb", [VOCAB, E], F32, isOutput=False)
    wqt_d = nc.declare_dram_parameter("wqt", [E, H * A], BF, isOutput=False)
    wkt_d = nc.declare_dram_parameter("wkt", [E, H * A], BF, isOutput=False)
    wvt_d = nc.declare_dram_parameter("wvt", [E, H * E], BF, isOutput=False)
    if use_beta:
        qb_d = nc.declare_dram_parameter("qb", [P, 4], F32, isOutput=False)
        kb_d = nc.declare_dram_parameter("kb", [P, 4], F32, isOutput=False)
        vb_d = nc.declare_dram_parameter("vb", [1, H * E], F32, isOutput=False)
    out_d = nc.declare_dram_parameter("out", [H, S, E], BF, isOutput=True)

    with tile.TileContext(nc) as tc:
        with tc.tile_pool(name="consts", bufs=1) as consts:
            idx_sb = consts.tile([P, S // 16], I16)
            nc.sync.dma_start(idx_sb[:], idx_d[:])
            wqt_sb = consts.tile([P, EC, H * A], BF)
            nc.sync.dma_start(
                wqt_sb[:], wqt_d.ap().rearrange("(ec p) j -> p ec j", p=P)
            )
            wvt_sb = consts.tile([P, EC, H * E], BF)
            nc.sync.dma_start(
                wvt_sb[:], wvt_d.ap().rearrange("(ec p) j -> p ec j", p=P)
            )
            wkt_sb = consts.tile([P, EC, H * A], BF)
            nc.sync.dma_start(
                wkt_sb[:], wkt_d.ap().rearrange("(ec p) j -> p ec j", p=P)
            )
            eps_sb = consts.tile([P, 1], F32)
            nc.vector.memset(eps_sb[:], LN_EPS)

            if use_beta:
                qb_sb = consts.tile([P, 4], F32)
                nc.sync.dma_start(qb_sb[:], qb_d[:])
                kb_sb = consts.tile([P, 4], F32)
                nc.sync.dma_start(kb_sb[:], kb_d[:])
                vb_sb = consts.tile([P, H * E], F32)
                vb_bcast = bass.AP(
                    tensor=vb_d, offset=0, ap=[[0, P], [1, H * E]]
                )
                nc.sync.dma_start(vb_sb[:], vb_bcast)

            # persistent activations
            hT_sb = consts.tile([P, EC, S], BF)       # hT[e%128, e//128, s]
            qT_sb = consts.tile([P, EC, S], BF)       # qT[ha%128, ha//128, s]
            kT_sb = consts.tile([P, EC, S], BF)
            vh_sb = consts.tile([P, NCH, H * E], BF)  # vh[p, c, v] = V[8p+c, v]

            # ------- phase A: gather + LN + transpose + vhat + proj -------
            with (
                tc.tile_pool(name="e_pool", bufs=1) as e_pool,
                tc.tile_pool(name="h_pool", bufs=3) as h_pool,
                tc.tile_pool(name="st_pool", bufs=8) as st_pool,
                tc.tile_pool(name="proj_psum", bufs=3, space="PSUM") as proj_psum,
                tc.tile_pool(name="warm_psum", bufs=1, space="PSUM") as warm_psum,
            ):
                # PE warm-up + filler: HAM un-throttles after ~3.4us of
                # sustained matmul activity, and the gather pipeline keeps
                # PE data-starved for ~30us — burn wqt-data matmuls so the
                # clock is at 2.4GHz when real work (proj) arrives.
                warm = warm_psum.tile([P, 512], F32, tag="warm")
                for wi in range(88):
                    nc.tensor.matmul(
                        warm[:],
                        wqt_sb[:, wi % EC, 0:P],
                        wqt_sb[:, wi % EC, 0:512],
                        start=True, stop=True, skip_group_check=True,
                    )

                def proj_half(nn):
                    # q/k projections for token half nn (needs hT chunks
                    # 4*nn .. 4*nn+3 only)
                    for w_sb, t_sb, bname in (
                        (wqt_sb, qT_sb, "qb"),
                        (wkt_sb, kT_sb, "kb"),
                    ):
                        for sl in range(4):
                            pq = proj_psum.tile([P, 512], F32, tag="pp")
                            for ec in range(EC):
                                nc.tensor.matmul(
                                    pq[:],
                                    w_sb[:, ec, sl * P : (sl + 1) * P],
                                    hT_sb[:, ec, nn * 512 : (nn + 1) * 512],
                                    start=(ec == 0), stop=(ec == EC - 1),
                                )
                            if use_beta:
                                bb = qb_sb if bname == "qb" else kb_sb
                                nc.vector.tensor_scalar_add(
                                    out=pq[:], in0=pq[:],
                                    scalar1=bb[:, sl : sl + 1],
                                )
                            nc.any.tensor_copy(
                                t_sb[:, sl, nn * 512 : (nn + 1) * 512], pq[:]
                            )

                for c in range(NCH):
                    e_t = e_pool.tile([P, 1, E], F32, tag=f"e{c}")
                    nc.gpsimd.dma_gather(
                        e_t[:], emb_d.ap(), idx_sb[:, 8 * c : 8 * (c + 1)],
                        P, P, E,
                    )
                    stt = st_pool.tile([P, 6], F32, tag="bn")
                    nc.vector.bn_stats(stt[:], e_t[:, 0, :])
                    mv = st_pool.tile([P, 2], F32, tag=f"mv{c}")
                    nc.vector.bn_aggr(mv[:], stt[:])
                    # inv-std: 1/sqrt(var+eps)
                    nc.scalar.activation(
                        out=mv[:, 1:2], in_=mv[:, 1:2],
                        func=mybir.ActivationFunctionType.Sqrt,
                        bias=eps_sb[:, 0:1], scale=1.0,
                    )
                    nc.vector.reciprocal(mv[:, 1:2], mv[:, 1:2])
                    h_t = h_pool.tile([P, E], BF)
                    nc.vector.tensor_scalar(
                        out=h_t[:], in0=e_t[:, 0, :],
                        scalar1=mv[:, 0:1], scalar2=mv[:, 1:2],
                        op0=mybir.AluOpType.subtract, op1=mybir.AluOpType.mult,
                    )
                    # XBAR transpose: hT[e%128, e//128, c*128+q] = h[q, e]
                    nc.sync.dma_start_transpose(
                        hT_sb[:, :, c * P : (c + 1) * P], h_t[:]
                    )
                    if c == 3:
                        proj_half(0)
                proj_half(1)

            # ---------------- phase C: attention ----------------
            with (
                tc.tile_pool(name="sc_psum", bufs=3, space="PSUM") as sc_psum,
                tc.tile_pool(name="out_psum", bufs=2, space="PSUM") as out_psum,
                tc.tile_pool(name="sstat", bufs=8) as sstat,
                tc.tile_pool(name="w_pool", bufs=3) as w_pool,
                tc.tile_pool(name="wraw_pool", bufs=5) as wraw_pool,
                tc.tile_pool(name="wt_pool", bufs=10) as wt_pool,
                tc.tile_pool(name="ob_pool", bufs=3) as ob_pool,
            ):
                def vhat_group(c):
                    # V-hat projection for t-group c, interleaved into phase
                    # C units 0..7 so the stats/sigmoid pipeline of early
                    # units overlaps this PE-heavy block.
                    for vp in range(4):
                        pv = sc_psum.tile([P, S], F32, tag="sc")
                        for ec in range(EC):
                            lhsT = hT_sb[:, ec, c * P : (c + 1) * P]
                            for nn in range(2):
                                lo = vp * 1024 + nn * 512
                                nc.tensor.matmul(
                                    pv[:, nn * 512 : (nn + 1) * 512],
                                    lhsT,
                                    wvt_sb[:, ec, lo : lo + 512],
                                    start=(ec == 0), stop=(ec == EC - 1),
                                )
                        nc.scalar.copy(
                            vh_sb[:, c, vp * 1024 : (vp + 1) * 1024], pv[:]
                        )

                def out_stage(args):
                    hp_, i_, wt2 = args
                    ob2 = ob_pool.tile([P, 2, E], BF, tag="ob")
                    for sub in range(2):
                        po = out_psum.tile([P, E], F32, tag="po")
                        for cc in range(NCH):
                            nc.tensor.matmul(
                                po[:],
                                wt2[:, sub * NCH + cc, :],
                                vh_sb[:, cc, (2 * hp_ + sub) * E
                                      : (2 * hp_ + sub + 1) * E],
                                start=(cc == 0), stop=(cc == NCH - 1),
                            )
                        nc.scalar.copy(ob2[:, sub, :], po[:])
                    # both heads in one DMA: dram [h, s, e] iterated s,h,e
                    nc.sync.dma_start(
                        out_d[2 * hp_ : 2 * hp_ + 2,
                              i_ * P : (i_ + 1) * P, :]
                        .rearrange("h s e -> s h e"),
                        ob2[:],
                    )

                pending = []
                unit_no = 0
                for hp in range(4):      # head pair (2hp, 2hp+1)
                    for i in range(NCH):  # query chunk
                        if unit_no < NCH:
                            vhat_group(unit_no)
                        unit_no += 1
                        psA = sc_psum.tile([P, S], F32, tag="sc")
                        psB = sc_psum.tile([P, S], F32, tag="sc")
                        ps = [psA, psB]
                        # interleave (sub, nn) so the two heads' K=64 MMs
                        # occupy different PE row groups concurrently
                        for nn in range(2):
                            for sub in range(2):
                                p0 = sub * 64
                                nc.tensor.matmul(
                                    ps[sub][:, nn * 512 : (nn + 1) * 512],
                                    qT_sb[p0 : p0 + 64, hp, i * P : (i + 1) * P],
                                    kT_sb[p0 : p0 + 64, hp,
                                          nn * 512 : (nn + 1) * 512],
                                    start=True, stop=True,
                                )
                        stmn = sstat.tile([P, 2], F32, tag="stmn")
                        stmx = sstat.tile([P, 2], F32, tag="stmx")
                        sta = sstat.tile([P, 2], F32, tag="sta")
                        stb = sstat.tile([P, 2], F32, tag="stb")
                        wraws = []
                        for sub in range(2):
                            wraw = wraw_pool.tile([P, S], BF, tag="wr")
                            # fused PSUM->SBUF copy + row-max accum (1x rate,
                            # PSUM read port bound)
                            nc.vector.tensor_scalar(
                                out=wraw[:], in0=ps[sub][:],
                                scalar1=-3.0e38, scalar2=None,
                                op0=mybir.AluOpType.max,
                                op1=mybir.AluOpType.max,
                                accum_out=stmx[:, sub : sub + 1],
                            )
                            # row-min via a second bf16 SBUF pass: tensor_scalar
                            # runs in 4x mode on bf16 (tensor_reduce does not)
                            wr2 = wraw_pool.tile([P, S], BF, tag="wr2")
                            nc.vector.tensor_scalar(
                                out=wr2[:], in0=wraw[:],
                                scalar1=3.0e38, scalar2=None,
                                op0=mybir.AluOpType.min,
                                op1=mybir.AluOpType.min,
                                accum_out=stmn[:, sub : sub + 1],
                            )
                            wraws.append(wraw)
                        # alpha = 10/(mx-mn); beta = -mn*alpha - 5
                        nc.vector.tensor_sub(sta[:], stmx[:], stmn[:])
                        nc.vector.reciprocal(sta[:], sta[:])
                        nc.vector.tensor_scalar_mul(sta[:], sta[:], 10.0)
                        nc.vector.tensor_mul(stb[:], stmn[:], sta[:])
                        nc.vector.tensor_scalar(
                            out=stb[:], in0=stb[:], scalar1=-1.0, scalar2=-5.0,
                            op0=mybir.AluOpType.mult, op1=mybir.AluOpType.add,
                        )
                        w_t = w_pool.tile([P, 2, S], BF, tag="w")
                        for sub in range(2):
                            nc.scalar.activation(
                                out=w_t[:, sub, :], in_=wraws[sub][:],
                                func=mybir.ActivationFunctionType.Sigmoid,
                                bias=stb[:, sub : sub + 1],
                                scale=sta[:, sub : sub + 1],
                            )
                        # one fused XBAR transpose for both heads:
                        # wt2[p, sub*NCH+c, q] = w_t[q, sub, c*128+p]
                        wt2 = wt_pool.tile([P, 2 * NCH, P], BF, tag="wt")
                        nc.sync.dma_start_transpose(wt2[:], w_t[:])
                        pending.append((hp, i, wt2))
                        if len(pending) > 8:
                            out_stage(pending.pop(0))
                for pp_ in pending:
                    out_stage(pp_)

    nc.compile()
    _BUILD_CACHE[(use_beta,)] = nc
    return nc


def _prep_inputs(x, emb_table, gamma, beta, Wq, Wk, Wv, use_beta):
    x = np.asarray(x)
    gamma = np.asarray(gamma, dtype=np.float32)
    beta = np.asarray(beta, dtype=np.float32)
    Wq = np.asarray(Wq, dtype=np.float32)
    Wk = np.asarray(Wk, dtype=np.float32)
    Wv = np.asarray(Wv, dtype=np.float32)
    emb = np.ascontiguousarray(np.asarray(emb_table, dtype=np.float32))

    # W'[h,a,e] = W[h,a,e] * gamma[e]; layouts [e, h*ad+a]
    wqt = np.ascontiguousarray(
        (Wq * gamma[None, None, :]).reshape(H * A, E).T.astype(BF16)
    )
    wkt = np.ascontiguousarray(
        (Wk * gamma[None, None, :]).reshape(H * A, E).T.astype(BF16)
    )
    wvt = np.ascontiguousarray(
        (Wv * gamma[None, None, :]).reshape(H * E, E).T.astype(BF16)
    )

    consts = dict(emb=emb, wqt=wqt, wkt=wkt, wvt=wvt)
    if use_beta:
        qb = (Wq.reshape(H * A, E) @ beta).astype(np.float32)   # [512]
        kb = (Wk.reshape(H * A, E) @ beta).astype(np.float32)
        vb = (Wv.reshape(H * E, E) @ beta).astype(np.float32)   # [4096]
        consts["qb"] = np.ascontiguousarray(qb.reshape(4, P).T)
        consts["kb"] = np.ascontiguousarray(kb.reshape(4, P).T)
        consts["vb"] = vb.reshape(1, H * E)

    in_maps = []
    for b in range(B):
        xi = x[b].astype(np.int64)
        idx16 = np.ascontiguousarray(
            xi.reshape(S // 16, 16).T.astype(np.int16)
        )  # [16, 64]; token j of chunk c sits at [j%16, 8c + j//16]
        idx_full = np.ascontiguousarray(np.tile(idx16, (8, 1)))  # [128, 64]
        in_maps.append(dict(idx=idx_full, **consts))
    return in_maps


def kernel(x, emb_table, gamma, beta, Wq, Wk, Wv):
    global LAST_RESULTS
    beta_arr = np.asarray(beta, dtype=np.float32)
    use_beta = bool(np.any(beta_arr != 0.0))

    nc = build_nc(use_beta)
    in_maps = _prep_inputs(x, emb_table, gamma, beta, Wq, Wk, Wv, use_beta)

    trace = os.environ.get("KERNEL_TRACE", "0") == "1"
    res = run_bass_kernel_spmd(
        nc, in_maps, core_ids=list(range(B)), trace=trace
    )
    LAST_RESULTS = res

    out = np.stack([np.asarray(res.results[b]["out"]) for b in range(B)], axis=0)
    return out.astype(np.float32)


if __name__ == "__main__":
    rng = np.random.default_rng(0)
    x = rng.integers(0, VOCAB, size=(B, S), dtype=np.int32)
    emb = rng.standard_normal((VOCAB, E), dtype=np.float32)
    gamma = np.ones(E, np.float32)
    beta = np.zeros(E, np.float32)
    Wq = rng.random((H, A, E), dtype=np.float32)
    Wk = rng.random((H, A, E), dtype=np.float32)
    Wv = rng.random((H, E, E), dtype=np.float32)
    out = kernel(x, emb, gamma, beta, Wq, Wk, Wv)
    print(out.shape, out.dtype)
